# revision 32
# baseline (speedup 1.0000x reference)
"""Trainium2 Bass kernel for 3-layer GATv2 + GraphNorm + attentional pooling.

Self-contained: accepts FULL inputs, shards across 8 NeuronCores internally,
returns FULL [64, 128] output.

Strategy: nodes partitioned 2000/core (padded to 2048); edges partitioned by
dst and grouped into 16 dst-blocks of 128 nodes per core. Per layer: sharded
x@W matmuls -> AllGather of the xl row-table -> edge phase (indirect-DMA row
gathers + mask matmuls accumulating transposed per-block outputs in PSUM)
-> GraphNorm (channel-partition orientation; tiny AllReduce of sums) -> ReLU.
Final attentional pooling via graph-mask matmuls + one small AllReduce.
"""

import numpy as np

import concourse.bass as bass
import concourse.bacc as bacc
import concourse.mybir as mybir
import concourse.tile as tile

F32 = mybir.dt.float32
I32 = mybir.dt.int32

NCORES = 8
N = 16000
E = 256000
B = 64
H = 4
NPC = 2000           # real nodes per core
NPAD = 2048          # padded nodes per core
NTOT = NPAD * NCORES
NBLK = 16            # dst blocks of 128 per core
NT = 16              # node tiles per core

# layer configs: (din, din_padded, HC, C)
LAYERS = [
    (960, 1024, 512, 128),
    (512, 512, 256, 64),
    (256, 256, 128, 32),
]

RG = [list(range(NCORES))]


# ---------------------------------------------------------------- host prep

def _host_prep(inputs):
    x = np.asarray(inputs["x"], np.float32)
    ei = np.asarray(inputs["edge_index"])
    batch = np.asarray(inputs["batch"])

    src = np.concatenate([ei[0], np.arange(N)]).astype(np.int64)
    dst = np.concatenate([ei[1], np.arange(N)]).astype(np.int64)
    gp_src = (src // NPC) * NPAD + (src % NPC)      # global padded index

    core = dst // NPC
    dloc = dst % NPC
    blk = dloc // 128

    # per (core, block) edge counts -> uniform tile count
    cnt = np.zeros((NCORES, NBLK), np.int64)
    np.add.at(cnt, (core, blk), 1)
    t_blk = int(np.ceil(cnt.max() / 128))
    cap = t_blk * 128

    srcg = np.zeros((NCORES, NBLK, cap), np.int32)
    dstg = np.zeros((NCORES, NBLK, cap), np.int32)
    dstc = np.full((NCORES, NBLK, cap), 300.0, np.float32)
    vald = np.zeros((NCORES, NBLK, cap), np.float32)

    order = np.lexsort((dloc, blk, core))
    so_core, so_blk, so_dloc, so_src = core[order], blk[order], dloc[order], gp_src[order]
    # slot position within each (core, blk) group
    group_id = so_core * NBLK + so_blk
    start = np.zeros(NCORES * NBLK + 1, np.int64)
    np.add.at(start, group_id + 1, 1)
    start = np.cumsum(start)
    slot = np.arange(len(order)) - start[group_id]
    srcg[so_core, so_blk, slot] = so_src.astype(np.int32)
    dstg[so_core, so_blk, slot] = so_dloc.astype(np.int32)
    dstc[so_core, so_blk, slot] = (so_dloc % 128).astype(np.float32)
    vald[so_core, so_blk, slot] = 1.0

    # SBUF layout [128 lanes, NBLK * t_blk cols]: col = b*t_blk + j, lane p
    # holds edge slot j*128 + p of block b.
    def to_tiles(a):
        # [NCORES, NBLK, cap] -> [NCORES, NBLK, t_blk, 128] -> [NCORES, 128, NBLK*t_blk]
        a = a.reshape(NCORES, NBLK, t_blk, 128).transpose(0, 3, 1, 2)
        return np.ascontiguousarray(a.reshape(NCORES, 128, NBLK * t_blk))

    srcg_t, dstg_t, dstc_t, vald_t = map(to_tiles, (srcg, dstg, dstc, vald))

    # per-core x, transposed + padded: [din_pad, NPAD]
    xT0 = np.zeros((NCORES, LAYERS[0][1], NPAD), np.float32)
    for k in range(NCORES):
        xk = x[k * NPC:(k + 1) * NPC]              # [NPC, 960]
        xT0[k, :960, :NPC] = xk.T

    # batch / node validity tiles [128, NT]
    gbt = np.full((NCORES, 128, NT), 300.0, np.float32)
    nvt = np.zeros((NCORES, 128, NT), np.float32)
    for k in range(NCORES):
        bk = batch[k * NPC:(k + 1) * NPC].astype(np.float32)
        pad = np.full(NPAD - NPC, 300.0, np.float32)
        gbt[k] = np.concatenate([bk, pad]).reshape(NT, 128).T
        nvt[k] = np.concatenate([np.ones(NPC, np.float32),
                                 np.zeros(NPAD - NPC, np.float32)]).reshape(NT, 128).T

    per_core = dict(
        srcg=srcg_t, dstg=dstg_t, dstc=dstc_t, vald=vald_t, xT0=xT0,
        gbt=gbt, nvt=nvt,
    )

    # ---- shared (replicated) weight inputs
    shared = {}
    for li, (din, dinp, HC, C) in enumerate(LAYERS):
        i = li + 1
        CH = HC // 128
        Wl = np.zeros((dinp, HC), np.float32)
        Wr = np.zeros((dinp, HC), np.float32)
        Wl[:din] = np.asarray(inputs[f"Wl{i}"], np.float32)
        Wr[:din] = np.asarray(inputs[f"Wr{i}"], np.float32)
        shared[f"Wl{i}"] = Wl
        shared[f"Wr{i}"] = Wr
        att = np.asarray(inputs[f"att{i}"], np.float32).reshape(-1)   # [HC]
        shared[f"attB{i}"] = np.broadcast_to(att, (128, HC)).copy()
        for nm in ("b", "gnw", "gnb", "gnm"):
            v = np.asarray(inputs[f"{nm}{i}"], np.float32)
            shared[f"{nm}T{i}"] = np.ascontiguousarray(v.reshape(CH, 128).T)  # [128, CH]
        # head-selector for z broadcast: S[h, ch]=1 iff channel ch is in head h
        S = np.zeros((4, HC), np.float32)
        for ch in range(HC):
            S[ch // C, ch] = 1.0
        shared[f"S{i}"] = S

    shared["aW1"] = np.asarray(inputs["aW1"], np.float32)             # [128,128]
    shared["ab1B"] = np.broadcast_to(
        np.asarray(inputs["ab1"], np.float32), (128, 128)).copy()
    shared["aW2B"] = np.broadcast_to(
        np.asarray(inputs["aW2"], np.float32).reshape(-1), (128, 128)).copy()
    shared["ab2B"] = np.full((128, 1), float(np.asarray(inputs["ab2"]).reshape(-1)[0]),
                             np.float32)
    shared["iota128"] = np.broadcast_to(
        np.arange(128, dtype=np.float32), (128, 128)).copy()
    shared["iota64"] = np.broadcast_to(
        np.arange(64, dtype=np.float32), (128, 64)).copy()
    shared["ident"] = np.eye(128, dtype=np.float32)
    cv15 = np.ones((128, 128), np.float32)
    cv15[:, NPC - 15 * 128:] = 0.0                                    # cols 80..127
    shared["cv15"] = cv15
    shared["ones_col"] = np.ones((128, 1), np.float32)

    return per_core, shared, t_blk


# ---------------------------------------------------------------- program

def _build_program(t_blk):
    nc = bacc.Bacc(None, num_devices=NCORES)
    TB = t_blk

    # ---- kernel I/O
    def din(name, shape, dt=F32):
        return nc.dram_tensor(name, shape, dt, kind="ExternalInput")

    srcg_d = din("srcg", [128, NBLK * TB], I32)
    dstg_d = din("dstg", [128, NBLK * TB], I32)
    dstc_d = din("dstc", [128, NBLK * TB])
    vald_d = din("vald", [128, NBLK * TB])
    xT0_d = din("xT0", [LAYERS[0][1], NPAD])
    gbt_d = din("gbt", [128, NT])
    nvt_d = din("nvt", [128, NT])

    Wd = {}
    for li, (dn, dinp, HC, C) in enumerate(LAYERS):
        i = li + 1
        Wd[f"Wl{i}"] = din(f"Wl{i}", [dinp, HC])
        Wd[f"Wr{i}"] = din(f"Wr{i}", [dinp, HC])
        Wd[f"attB{i}"] = din(f"attB{i}", [128, HC])
        for nm in ("bT", "gnwT", "gnbT", "gnmT"):
            Wd[f"{nm}{i}"] = din(f"{nm[:-1]}T{i}", [128, HC // 128])
        Wd[f"S{i}"] = din(f"S{i}", [4, HC])
    aW1_d = din("aW1", [128, 128])
    ab1B_d = din("ab1B", [128, 128])
    aW2B_d = din("aW2B", [128, 128])
    ab2B_d = din("ab2B", [128, 1])
    iota128_d = din("iota128", [128, 128])
    iota64_d = din("iota64", [128, 64])
    ident_d = din("ident", [128, 128])
    cv15_d = din("cv15", [128, 128])
    ones_col_d = din("ones_col", [128, 1])

    out_d = nc.dram_tensor("out", [B, 128], F32, kind="ExternalOutput")
    import os
    DBG = bool(int(os.environ.get("BASSGAT_DEBUG", "0")))
    dbg = {}
    if DBG:
        dbg["xl1"] = nc.dram_tensor("dbg_xl1", [NPAD, 512], F32, kind="ExternalOutput")
        dbg["xr1"] = nc.dram_tensor("dbg_xr1", [NPAD, 512], F32, kind="ExternalOutput")
        dbg["tbl1"] = nc.dram_tensor("dbg_tbl1", [NPAD, 512], F32, kind="ExternalOutput")
        dbg["xt1pre"] = nc.dram_tensor("dbg_xt1pre", [512, NPAD], F32, kind="ExternalOutput")
        dbg["xt1post"] = nc.dram_tensor("dbg_xt1post", [512, NPAD], F32, kind="ExternalOutput")
        dbg["stats1"] = nc.dram_tensor("dbg_stats1", [128, 16], F32, kind="ExternalOutput")
        dbg["xt2pre"] = nc.dram_tensor("dbg_xt2pre", [256, NPAD], F32, kind="ExternalOutput")
        dbg["xt2post"] = nc.dram_tensor("dbg_xt2post", [256, NPAD], F32, kind="ExternalOutput")
        dbg["xt3pre"] = nc.dram_tensor("dbg_xt3pre", [128, NPAD], F32, kind="ExternalOutput")
        dbg["xt3post"] = nc.dram_tensor("dbg_xt3post", [128, NPAD], F32, kind="ExternalOutput")
        dbg["gate"] = nc.dram_tensor("dbg_gate", [128, NT], F32, kind="ExternalOutput")
        dbg["pgate"] = nc.dram_tensor("dbg_pgate", [128, NT], F32, kind="ExternalOutput")
        dbg["poolacc"] = nc.dram_tensor("dbg_poolacc", [B, 129], F32, kind="ExternalOutput")
        dbg["x3r0"] = nc.dram_tensor("dbg_x3r0", [128, 128], F32, kind="ExternalOutput")

    with tile.TileContext(nc) as tc:
        with (
            tc.tile_pool(name="dram", bufs=1, space="DRAM") as dram,
            tc.tile_pool(name="const", bufs=1) as cpool,
            tc.tile_pool(name="persist", bufs=1) as ppool,
            tc.tile_pool(name="work", bufs=3) as wpool,
            tc.tile_pool(name="small", bufs=4) as spool,
            tc.tile_pool(name="ps", bufs=2, space="PSUM") as ps,
        ):
            # ---------------- DRAM scratch
            tblL = {}
            tblR = {}
            agin = {}
            for li, (dn, dinp, HC, C) in enumerate(LAYERS):
                tblL[li] = dram.tile([NTOT, HC], F32, addr_space="Shared",
                                     name=f"tblL{li}")
                tblR[li] = dram.tile([NPAD, HC], F32, name=f"tblR{li}")
                agin[li] = dram.tile([NPAD, HC], F32, name=f"agin{li}")
            stats_in = {}
            stats_out = {}
            for li in range(3):
                stats_in[li] = dram.tile([128, 16], F32, name=f"stats_in{li}")
                stats_out[li] = dram.tile([128, 16], F32, addr_space="Shared",
                                          name=f"stats_out{li}")
            pool_in = dram.tile([B, 129], F32, name="pool_in")
            pool_out = dram.tile([B, 129], F32, addr_space="Shared",
                                 name="pool_out")

            # ---------------- constants / index tables to SBUF
            def cload(dten, shape, dt=F32, name="c"):
                t = cpool.tile(shape, dt, name=name)
                nc.sync.dma_start(t[:], dten[:])
                return t

            srcg = cload(srcg_d, [128, NBLK * TB], I32, "srcg_sb")
            dstg = cload(dstg_d, [128, NBLK * TB], I32, "dstg_sb")
            dstc = cload(dstc_d, [128, NBLK * TB], name="dstc_sb")
            vald = cload(vald_d, [128, NBLK * TB], name="vald_sb")
            gbt = cload(gbt_d, [128, NT], name="gbt_sb")
            nvt = cload(nvt_d, [128, NT], name="nvt_sb")
            iota128 = cload(iota128_d, [128, 128], name="iota128_sb")
            iota64 = cload(iota64_d, [128, 64], name="iota64_sb")
            ident = cload(ident_d, [128, 128], name="ident_sb")
            cv15 = cload(cv15_d, [128, 128], name="cv15_sb")
            ones_col = cload(ones_col_d, [128, 1], name="ones_sb")
            aW1 = cload(aW1_d, [128, 128], name="aW1_sb")
            ab1B = cload(ab1B_d, [128, 128], name="ab1B_sb")
            aW2B = cload(aW2B_d, [128, 128], name="aW2B_sb")
            ab2B = cload(ab2B_d, [128, 1], name="ab2B_sb")

            # xT chunks for layers 2,3 and pooling input, produced on device.
            # xTn[i][cc] : [128, NPAD] tile holding channels of layer i's input
            # (= layer i-1's output, HC_{i-1} channels).
            xTn = {1: [], 2: [], 3: []}
            for i in (1, 2, 3):
                chn = LAYERS[i - 1][2] // 128
                for cc in range(chn):
                    t = ppool.tile([128, NPAD], F32, name=f"xT{i}_{cc}")
                    xTn[i].append(t)

            ExpF = mybir.ActivationFunctionType.Exp
            PreluF = mybir.ActivationFunctionType.Prelu
            SquareF = mybir.ActivationFunctionType.Square
            SqrtF = mybir.ActivationFunctionType.Sqrt

            # ================= per-layer pipeline =================
            for li, (dn, dinp, HC, C) in enumerate(LAYERS):
                i = li + 1
                CH = HC // 128
                CHI = dinp // 128

                # ---- load weights (tag slots shared across layers, bufs=8 max CHI)
                wl_sb = []
                wr_sb = []
                for ci in range(CHI):
                    wl = wpool.tile([128, HC], F32, tag="wl", name=f"wl{i}_{ci}",
                                    bufs=8)
                    nc.sync.dma_start(wl[:], Wd[f"Wl{i}"][ci * 128:(ci + 1) * 128, :])
                    wl_sb.append(wl)
                    wr = wpool.tile([128, HC], F32, tag="wr", name=f"wr{i}_{ci}",
                                    bufs=8)
                    nc.sync.dma_start(wr[:], Wd[f"Wr{i}"][ci * 128:(ci + 1) * 128, :])
                    wr_sb.append(wr)
                attB = wpool.tile([128, HC], F32, tag="attB", name=f"attB{i}_sb",
                                  bufs=1)
                nc.sync.dma_start(attB[:], Wd[f"attB{i}"][:])
                bT = spool.tile([128, CH], F32, tag="bT", name=f"bT{i}_sb", bufs=1)
                nc.sync.dma_start(bT[:], Wd[f"bT{i}"][:])
                Ssel = spool.tile([4, HC], F32, tag="Ssel", name=f"S{i}_sb",
                                  bufs=1)
                nc.sync.dma_start(Ssel[:], Wd[f"S{i}"][:])

                # ---- matmul phase: xl/xr rows for all 16 node tiles.
                # L1 streams lhsT from DRAM in 2-node-tile groups; L2/L3 use
                # SBUF-resident xT chunks.
                def emit_rows(nt, ps_xl, ps_xr):
                    xl_sb = wpool.tile([128, HC], F32, tag="xlrow", name="xl_sb")
                    xr_sb = wpool.tile([128, HC], F32, tag="xrrow", name="xr_sb")
                    nc.vector.tensor_copy(xl_sb[:], ps_xl[:])
                    nc.vector.tensor_copy(xr_sb[:], ps_xr[:])
                    nc.sync.dma_start(agin[li][nt * 128:(nt + 1) * 128, :], xl_sb[:])
                    nc.sync.dma_start(tblR[li][nt * 128:(nt + 1) * 128, :], xr_sb[:])

                if li == 0:
                    for g in range(NT // 2):
                        pxl = [ps.tile([128, HC], F32, tag="big", name="ps_xl", bufs=4)
                               for _ in range(2)]
                        pxr = [ps.tile([128, HC], F32, tag="big", name="ps_xr", bufs=4)
                               for _ in range(2)]
                        for ci in range(CHI):
                            lhsg = wpool.tile([128, 256], F32, tag="xt0",
                                              name="xt0_t", bufs=4)
                            nc.sync.dma_start(
                                lhsg[:], xT0_d[ci * 128:(ci + 1) * 128,
                                               g * 256:(g + 1) * 256])
                            for t in range(2):
                                st = (ci == 0)
                                sp = (ci == CHI - 1)
                                nc.tensor.matmul(
                                    pxl[t][:], lhsT=lhsg[:, t * 128:(t + 1) * 128],
                                    rhs=wl_sb[ci][:], start=st, stop=sp)
                                nc.tensor.matmul(
                                    pxr[t][:], lhsT=lhsg[:, t * 128:(t + 1) * 128],
                                    rhs=wr_sb[ci][:], start=st, stop=sp)
                        for t in range(2):
                            emit_rows(g * 2 + t, pxl[t], pxr[t])
                else:
                    for nt in range(NT):
                        ps_xl = ps.tile([128, HC], F32, tag="big", name="ps_xl", bufs=4)
                        ps_xr = ps.tile([128, HC], F32, tag="big", name="ps_xr", bufs=4)
                        for ci in range(CHI):
                            lhs = xTn[li][ci][:, nt * 128:(nt + 1) * 128]
                            st = (ci == 0)
                            sp = (ci == CHI - 1)
                            nc.tensor.matmul(ps_xl[:], lhsT=lhs[:], rhs=wl_sb[ci][:],
                                             start=st, stop=sp)
                            nc.tensor.matmul(ps_xr[:], lhsT=lhs[:], rhs=wr_sb[ci][:],
                                             start=st, stop=sp)
                        emit_rows(nt, ps_xl, ps_xr)

                # ---- AllGather xl rows -> global table
                nc.gpsimd.collective_compute(
                    "AllGather", mybir.AluOpType.bypass, replica_groups=RG,
                    ins=[agin[li].opt()], outs=[tblL[li].opt()],
                )

                # ---- edge phase
                for b in range(NBLK):
                    acc_ps = [ps.tile([128, 128], F32, tag="big",
                                      name=f"acc_ps{cc}", bufs=4)
                              for cc in range(CH)]
                    z_ps = ps.tile([4, 128], F32, tag="z", name="z_ps", bufs=1)
                    for j in range(TB):
                        s = b * TB + j
                        st = (j == 0)
                        sp = (j == TB - 1)
                        g_l = wpool.tile([128, HC], F32, tag="gl", name="g_l")
                        nc.gpsimd.indirect_dma_start(
                            out=g_l[:], out_offset=None, in_=tblL[li][:],
                            in_offset=bass.IndirectOffsetOnAxis(
                                ap=srcg[:, s:s + 1], axis=0))
                        g_r = wpool.tile([128, HC], F32, tag="gr", name="g_r")
                        nc.gpsimd.indirect_dma_start(
                            out=g_r[:], out_offset=None, in_=tblR[li][:],
                            in_offset=bass.IndirectOffsetOnAxis(
                                ap=dstg[:, s:s + 1], axis=0))
                        tsum = wpool.tile([128, HC], F32, tag="tsum", name="tsum")
                        nc.vector.tensor_add(tsum[:], g_l[:], g_r[:])
                        lr = wpool.tile([128, HC], F32, tag="lr", name="lr")
                        nc.scalar.activation(lr[:], tsum[:], PreluF, alpha=0.2)
                        u = wpool.tile([128, HC], F32, tag="u", name="u")
                        nc.vector.tensor_mul(u[:], lr[:], attB[:])
                        logit = spool.tile([128, 4], F32, tag="logit", name="logit")
                        nc.vector.tensor_reduce(
                            logit[:], u[:].rearrange("p (h c) -> p h c", h=4),
                            axis=mybir.AxisListType.X, op=mybir.AluOpType.add)
                        pex = spool.tile([128, 4], F32, tag="pex", name="pex")
                        nc.scalar.activation(pex[:], logit[:], ExpF)
                        pv = spool.tile([128, 4], F32, tag="pv", name="pv")
                        nc.vector.tensor_scalar_mul(pv[:], pex[:], vald[:, s:s + 1])
                        mask = wpool.tile([128, 128], F32, tag="mask", name="mask")
                        nc.vector.tensor_tensor(
                            out=mask[:], in0=dstc[:, s:s + 1].to_broadcast([128, 128]),
                            in1=iota128[:], op=mybir.AluOpType.is_equal)
                        glp = wpool.tile([128, HC], F32, tag="glp", name="glp")
                        nc.vector.tensor_tensor(
                            out=glp[:].rearrange("p (h c) -> p h c", h=4),
                            in0=g_l[:].rearrange("p (h c) -> p h c", h=4),
                            in1=pv[:].to_broadcast([128, 4, C]),
                            op=mybir.AluOpType.mult)
                        nc.tensor.matmul(z_ps[:], lhsT=pv[:], rhs=mask[:],
                                         start=st, stop=sp)
                        for cc in range(CH):
                            nc.tensor.matmul(
                                acc_ps[cc][:],
                                lhsT=glp[:, cc * 128:(cc + 1) * 128], rhs=mask[:],
                                start=st, stop=sp)
                    # ---- post-block: z reciprocal, scale, bias
                    zrec = spool.tile([4, 128], F32, tag="zrec", name="zrec")
                    nc.vector.tensor_scalar_add(zrec[:], z_ps[:], 1e-16)
                    nc.vector.reciprocal(zrec[:], zrec[:])
                    for cc in range(CH):
                        zrb_ps = ps.tile([128, 128], F32, tag="zrb", name="zrb_ps",
                                         bufs=1)
                        nc.tensor.matmul(zrb_ps[:],
                                         lhsT=Ssel[:, cc * 128:(cc + 1) * 128],
                                         rhs=zrec[:], start=True, stop=True)
                        zrb = wpool.tile([128, 128], F32, tag="zrb_sb", name="zrb_sb")
                        nc.vector.tensor_copy(zrb[:], zrb_ps[:])
                        dstv = xTn[i][cc][:, b * 128:(b + 1) * 128]
                        nc.vector.tensor_mul(dstv, acc_ps[cc][:], zrb[:])
                        nc.vector.tensor_scalar_add(dstv, dstv, bT[:, cc:cc + 1])
                        if b == NBLK - 1:
                            nc.vector.tensor_mul(dstv, dstv, cv15[:])

                if DBG:
                    if li == 0:
                        nc.sync.dma_start(dbg["xl1"][:], agin[0][:])
                        nc.sync.dma_start(dbg["xr1"][:], tblR[0][:])
                        nc.sync.dma_start(dbg["tbl1"][:], tblL[0][0:NPAD, :])
                    for cc in range(CH):
                        nc.sync.dma_start(
                            dbg[f"xt{i}pre"][cc * 128:(cc + 1) * 128, :],
                            xTn[i][cc][:])

                # ---- GraphNorm stats: S1 at cols 0..CH-1, S2 at cols 8..8+CH-1
                stats = spool.tile([128, 16], F32, tag="stats", name="stats")
                nc.vector.memset(stats[:], 0.0)
                sq = wpool.tile([128, NPAD], F32, tag="sq", name="sq", bufs=1)
                for cc in range(CH):
                    nc.vector.tensor_reduce(
                        stats[:, cc:cc + 1], xTn[i][cc][:],
                        axis=mybir.AxisListType.X, op=mybir.AluOpType.add)
                    nc.scalar.activation(sq[:], xTn[i][cc][:], SquareF)
                    nc.vector.tensor_reduce(
                        stats[:, 8 + cc:8 + cc + 1], sq[:],
                        axis=mybir.AxisListType.X, op=mybir.AluOpType.add)
                nc.sync.dma_start(stats_in[li][:], stats[:])
                nc.gpsimd.collective_compute(
                    "AllReduce", mybir.AluOpType.add, replica_groups=RG,
                    ins=[stats_in[li].opt()], outs=[stats_out[li].opt()],
                )
                gstats = spool.tile([128, 16], F32, tag="gstats", name="gstats")
                nc.sync.dma_start(gstats[:], stats_out[li][:])

                gnw = spool.tile([128, CH], F32, tag="gnw", name="gnw")
                gnb = spool.tile([128, CH], F32, tag="gnb", name="gnb")
                gnm = spool.tile([128, CH], F32, tag="gnm", name="gnm")
                nc.sync.dma_start(gnw[:], Wd[f"gnwT{i}"][:])
                nc.sync.dma_start(gnb[:], Wd[f"gnbT{i}"][:])
                nc.sync.dma_start(gnm[:], Wd[f"gnmT{i}"][:])

                mn = spool.tile([128, CH], F32, tag="mn", name="mn")
                e2 = spool.tile([128, CH], F32, tag="e2", name="e2")
                nc.vector.tensor_scalar_mul(mn[:], gstats[:, :CH], 1.0 / N)
                nc.vector.tensor_scalar_mul(e2[:], gstats[:, 8:8 + CH], 1.0 / N)
                msm = spool.tile([128, CH], F32, tag="msm", name="msm")
                nc.vector.tensor_mul(msm[:], mn[:], gnm[:])
                t1 = spool.tile([128, CH], F32, tag="t1", name="t1")
                nc.vector.tensor_scalar_mul(t1[:], mn[:], 2.0)
                nc.vector.tensor_sub(t1[:], t1[:], msm[:])
                nc.vector.tensor_mul(t1[:], t1[:], msm[:])
                var = spool.tile([128, CH], F32, tag="var", name="var")
                nc.vector.tensor_sub(var[:], e2[:], t1[:])
                nc.vector.tensor_scalar_add(var[:], var[:], 1e-5)
                sd = spool.tile([128, CH], F32, tag="sd", name="sd")
                nc.scalar.activation(sd[:], var[:], SqrtF)
                istd = spool.tile([128, CH], F32, tag="istd", name="istd")
                nc.vector.reciprocal(istd[:], sd[:])
                A = spool.tile([128, CH], F32, tag="A", name="A")
                nc.vector.tensor_mul(A[:], gnw[:], istd[:])
                B2 = spool.tile([128, CH], F32, tag="B2", name="B2")
                nc.vector.tensor_mul(B2[:], A[:], msm[:])
                nc.vector.tensor_sub(B2[:], gnb[:], B2[:])
                for cc in range(CH):
                    nc.vector.tensor_scalar(
                        out=xTn[i][cc][:], in0=xTn[i][cc][:],
                        scalar1=A[:, cc:cc + 1], scalar2=B2[:, cc:cc + 1],
                        op0=mybir.AluOpType.mult, op1=mybir.AluOpType.add)
                    nc.vector.tensor_scalar_max(xTn[i][cc][:], xTn[i][cc][:], 0.0)
                if DBG:
                    if li == 0:
                        nc.sync.dma_start(dbg["stats1"][:], stats_out[0][:])
                    for cc in range(CH):
                        nc.sync.dma_start(
                            dbg[f"xt{i}post"][cc * 128:(cc + 1) * 128, :],
                            xTn[i][cc][:])

            # ================= pooling =================
            x3 = xTn[3][0]
            # rows of x3 via PE transpose
            x3r = []
            for ntl in range(NT):
                tps = ps.tile([128, 128], F32, tag="big", name="tps", bufs=4)
                nc.tensor.transpose(tps[:], x3[:, ntl * 128:(ntl + 1) * 128], ident[:])
                xr_t = ppool.tile([128, 128], F32, name=f"x3r_{ntl}")
                nc.vector.tensor_copy(xr_t[:], tps[:])
                x3r.append(xr_t)

            gate = spool.tile([128, NT], F32, tag="gate", name="gate")
            for ntl in range(NT):
                h_ps = ps.tile([128, 128], F32, tag="big", name="h_ps", bufs=4)
                nc.tensor.matmul(h_ps[:], lhsT=x3[:, ntl * 128:(ntl + 1) * 128],
                                 rhs=aW1[:], start=True, stop=True)
                hsb = wpool.tile([128, 128], F32, tag="hsb", name="hsb")
                nc.vector.tensor_add(hsb[:], h_ps[:], ab1B[:])
                nc.vector.tensor_scalar_max(hsb[:], hsb[:], 0.0)
                nc.vector.tensor_mul(hsb[:], hsb[:], aW2B[:])
                nc.vector.tensor_reduce(
                    gate[:, ntl:ntl + 1], hsb[:],
                    axis=mybir.AxisListType.X, op=mybir.AluOpType.add)
            nc.vector.tensor_scalar_add(gate[:], gate[:], ab2B[:, 0:1])
            pgate = spool.tile([128, NT], F32, tag="pgate", name="pgate")
            nc.scalar.activation(pgate[:], gate[:], ExpF)
            nc.vector.tensor_mul(pgate[:], pgate[:], nvt[:])
            if DBG:
                nc.sync.dma_start(dbg["gate"][:], gate[:])
                nc.sync.dma_start(dbg["pgate"][:], pgate[:])
                nc.sync.dma_start(dbg["x3r0"][:], x3r[0][:])

            pool_ps = ps.tile([B, 128], F32, tag="z", name="pool_ps", bufs=1)
            poolz_ps = ps.tile([B, 1], F32, tag="zrb", name="poolz_ps", bufs=1)
            for ntl in range(NT):
                mb = wpool.tile([128, B], F32, tag="mb", name="mb")
                nc.vector.tensor_tensor(
                    out=mb[:], in0=gbt[:, ntl:ntl + 1].to_broadcast([128, B]),
                    in1=iota64[:], op=mybir.AluOpType.is_equal)
                nc.vector.tensor_scalar_mul(mb[:], mb[:], pgate[:, ntl:ntl + 1])
                st = (ntl == 0)
                sp = (ntl == NT - 1)
                nc.tensor.matmul(pool_ps[:], lhsT=mb[:], rhs=x3r[ntl][:],
                                 start=st, stop=sp)
                nc.tensor.matmul(poolz_ps[:], lhsT=mb[:], rhs=ones_col[:],
                                 start=st, stop=sp)
            pool_sb = spool.tile([B, 129], F32, tag="poolsb", name="pool_sb")
            nc.vector.tensor_copy(pool_sb[:, :128], pool_ps[:])
            nc.vector.tensor_copy(pool_sb[:, 128:129], poolz_ps[:])
            nc.sync.dma_start(pool_in[:], pool_sb[:])
            nc.gpsimd.collective_compute(
                "AllReduce", mybir.AluOpType.add, replica_groups=RG,
                ins=[pool_in.opt()], outs=[pool_out.opt()],
            )
            poolg = spool.tile([B, 129], F32, tag="poolg", name="poolg")
            nc.sync.dma_start(poolg[:], pool_out[:])
            if DBG:
                nc.sync.dma_start(dbg["poolacc"][:], pool_in[:])
            zg = spool.tile([B, 1], F32, tag="zg", name="zg")
            nc.vector.tensor_scalar_add(zg[:], poolg[:, 128:129], 1e-16)
            nc.vector.reciprocal(zg[:], zg[:])
            outv = spool.tile([B, 128], F32, tag="outv", name="outv")
            nc.vector.tensor_scalar_mul(outv[:], poolg[:, :128], zg[:, 0:1])
            nc.sync.dma_start(out_d[:], outv[:])

    nc.finalize()
    return nc


# ---------------------------------------------------------------- runner

_CACHE = {}


def _get_runner(t_blk):
    if t_blk in _CACHE:
        return _CACHE[t_blk]
    nc = _build_program(t_blk)
    _CACHE[t_blk] = nc
    return nc


def kernel(**inputs):
    from concourse.bass_utils import run_bass_kernel_spmd

    per_core, shared, t_blk = _host_prep(inputs)
    nc = _get_runner(t_blk)

    in_maps = []
    for k in range(NCORES):
        m = {
            "srcg": per_core["srcg"][k], "dstg": per_core["dstg"][k],
            "dstc": per_core["dstc"][k], "vald": per_core["vald"][k],
            "xT0": per_core["xT0"][k], "gbt": per_core["gbt"][k],
            "nvt": per_core["nvt"][k],
        }
        m.update(shared)
        in_maps.append(m)

    res = run_bass_kernel_spmd(nc, in_maps, list(range(NCORES)))
    global LAST_RESULTS
    LAST_RESULTS = res.results
    return res.results[0]["out"]


if __name__ == "__main__":
    import reference
    inputs = {k: np.asarray(v) for k, v in reference.setup_inputs().items()}
    out = kernel(**inputs)
    exp = np.asarray(reference.reference(**inputs))
    err = np.abs(out - exp).max() / (np.abs(exp).max() + 1e-12)
    print("rel err:", err)


# revision 40
# speedup vs baseline: 1122.4129x; 1122.4129x over previous
"""Trainium2 Bass kernel for 3-layer GATv2 + GraphNorm + attentional pooling.

Self-contained: accepts FULL inputs, shards across 8 NeuronCores internally,
returns FULL [64, 128] output.

Strategy: nodes partitioned 2000/core (padded to 2048); edges partitioned by
dst and grouped into 16 dst-blocks of 128 nodes per core. Per layer: sharded
x@W matmuls (bf16) -> AllGather of the xl row-table -> edge phase:
  - xl[src] rows fetched per 128-edge tile with one indirect DMA (bf16 rows)
  - xr[dst] expanded on-chip per tile via PE (maskT.T @ xr_block) - no gather
  - GATv2 logit chain batched over half-blocks on DVE/ACT (Prelu+Exp share
    one ACT table set)
  - softmax-z and message aggregation via mask matmuls accumulating
    transposed per-block outputs in PSUM; z folded in as a per-dst-column
    reciprocal scale afterward (logits are bounded, no max-subtraction)
-> GraphNorm in channel-partition orientation (tiny AllReduce) -> ReLU.
Final attentional pooling via graph-mask matmuls + one small AllReduce.
"""

import os
import numpy as np
import ml_dtypes

import concourse.bass as bass
import concourse.bacc as bacc
import concourse.mybir as mybir
import concourse.tile as tile

F32 = mybir.dt.float32
BF16 = mybir.dt.bfloat16
I32 = mybir.dt.int32
NPBF = ml_dtypes.bfloat16

NCORES = 8
N = 16000
E = 256000
B = 64
H = 4
NPC = 2000           # real nodes per core
NPAD = 2048          # padded nodes per core
NTOT = NPAD * NCORES
NBLK = 16            # dst blocks of 128 per core
NT = 16              # node tiles per core

# layer configs: (din, din_padded, HC, C)
LAYERS = [
    (960, 1024, 512, 128),
    (512, 512, 256, 64),
    (256, 256, 128, 32),
]

RG = [list(range(NCORES))]


# ---------------------------------------------------------------- host prep

def _host_prep(inputs):
    x = np.asarray(inputs["x"], np.float32)
    ei = np.asarray(inputs["edge_index"])
    batch = np.asarray(inputs["batch"])

    src = np.concatenate([ei[0], np.arange(N)]).astype(np.int64)
    dst = np.concatenate([ei[1], np.arange(N)]).astype(np.int64)
    gp_src = (src // NPC) * NPAD + (src % NPC)      # global padded index

    core = dst // NPC
    dloc = dst % NPC
    blk = dloc // 128

    cnt = np.zeros((NCORES, NBLK), np.int64)
    np.add.at(cnt, (core, blk), 1)
    t_blk = int(np.ceil(cnt.max() / 128))
    cap = t_blk * 128

    srcg = np.zeros((NCORES, NBLK, cap), np.int32)
    dstc = np.full((NCORES, NBLK, cap), 300.0, np.float32)
    vald = np.zeros((NCORES, NBLK, cap), np.float32)

    order = np.lexsort((dloc, blk, core))
    so_core, so_blk, so_dloc, so_src = core[order], blk[order], dloc[order], gp_src[order]
    group_id = so_core * NBLK + so_blk
    start = np.zeros(NCORES * NBLK + 1, np.int64)
    np.add.at(start, group_id + 1, 1)
    start = np.cumsum(start)
    slot = np.arange(len(order)) - start[group_id]
    srcg[so_core, so_blk, slot] = so_src.astype(np.int32)
    dstc[so_core, so_blk, slot] = (so_dloc % 128).astype(np.float32)
    vald[so_core, so_blk, slot] = 1.0

    # SBUF layout [128 lanes, NBLK * t_blk cols]: col = b*t_blk + j, lane p
    # holds edge slot j*128 + p of block b.
    def to_tiles(a):
        a = a.reshape(NCORES, NBLK, t_blk, 128).transpose(0, 3, 1, 2)
        return np.ascontiguousarray(a.reshape(NCORES, 128, NBLK * t_blk))

    srcg_t, dstc_t, vald_t = map(to_tiles, (srcg, dstc, vald))

    xT0 = np.zeros((NCORES, LAYERS[0][1], NPAD), NPBF)
    for k in range(NCORES):
        xk = x[k * NPC:(k + 1) * NPC]
        xT0[k, :960, :NPC] = xk.T.astype(NPBF)

    gbt = np.full((NCORES, 128, NT), 300.0, np.float32)
    nvt = np.zeros((NCORES, 128, NT), np.float32)
    for k in range(NCORES):
        bk = batch[k * NPC:(k + 1) * NPC].astype(np.float32)
        pad = np.full(NPAD - NPC, 300.0, np.float32)
        gbt[k] = np.concatenate([bk, pad]).reshape(NT, 128).T
        nvt[k] = np.concatenate([np.ones(NPC, np.float32),
                                 np.zeros(NPAD - NPC, np.float32)]).reshape(NT, 128).T

    per_core = dict(srcg=srcg_t, dstc=dstc_t, vald=vald_t, xT0=xT0,
                    gbt=gbt, nvt=nvt)

    shared = {}
    for li, (din, dinp, HC, C) in enumerate(LAYERS):
        i = li + 1
        CH = HC // 128
        Wl = np.zeros((dinp, HC), NPBF)
        Wr = np.zeros((dinp, HC), NPBF)
        Wl[:din] = np.asarray(inputs[f"Wl{i}"], np.float32).astype(NPBF)
        Wr[:din] = np.asarray(inputs[f"Wr{i}"], np.float32).astype(NPBF)
        shared[f"Wl{i}"] = Wl
        shared[f"Wr{i}"] = Wr
        att = np.asarray(inputs[f"att{i}"], np.float32).reshape(-1)
        shared[f"attB{i}"] = np.broadcast_to(att, (128, HC)).copy()
        for nm in ("b", "gnw", "gnb", "gnm"):
            v = np.asarray(inputs[f"{nm}{i}"], np.float32)
            shared[f"{nm}T{i}"] = np.ascontiguousarray(v.reshape(CH, 128).T)
        S = np.zeros((4, HC), np.float32)
        for ch in range(HC):
            S[ch // C, ch] = 1.0
        shared[f"S{i}"] = S

    shared["aW1"] = np.asarray(inputs["aW1"], np.float32).astype(NPBF)
    shared["ab1B"] = np.broadcast_to(
        np.asarray(inputs["ab1"], np.float32), (128, 128)).copy()
    shared["aW2B"] = np.broadcast_to(
        np.asarray(inputs["aW2"], np.float32).reshape(-1), (128, 128)).copy()
    shared["ab2B"] = np.full((128, 1), float(np.asarray(inputs["ab2"]).reshape(-1)[0]),
                             np.float32)
    shared["iota128"] = np.broadcast_to(
        np.arange(128, dtype=np.float32), (128, 128)).copy()
    shared["iota64"] = np.broadcast_to(
        np.arange(64, dtype=np.float32), (128, 64)).copy()
    shared["ident"] = np.eye(128, dtype=np.float32)
    cv15 = np.ones((128, 128), np.float32)
    cv15[:, NPC - 15 * 128:] = 0.0
    shared["cv15"] = cv15
    shared["ones_col"] = np.ones((128, 1), NPBF)

    return per_core, shared, t_blk


# ---------------------------------------------------------------- program

def _build_program(t_blk):
    nc = bacc.Bacc(None, num_devices=NCORES)
    TB = t_blk
    TBH = (TB + 1) // 2          # half-block width
    HALVES = [(0, TBH), (TBH, TB - TBH)]
    EDGE_REPS = int(os.environ.get("BASSGAT_EDGE_REPS", "1"))

    def din(name, shape, dt=F32):
        return nc.dram_tensor(name, shape, dt, kind="ExternalInput")

    srcg_d = din("srcg", [128, NBLK * TB], I32)
    dstc_d = din("dstc", [128, NBLK * TB])
    vald_d = din("vald", [128, NBLK * TB])
    xT0_d = din("xT0", [LAYERS[0][1], NPAD], BF16)
    gbt_d = din("gbt", [128, NT])
    nvt_d = din("nvt", [128, NT])

    Wd = {}
    for li, (dn, dinp, HC, C) in enumerate(LAYERS):
        i = li + 1
        Wd[f"Wl{i}"] = din(f"Wl{i}", [dinp, HC], BF16)
        Wd[f"Wr{i}"] = din(f"Wr{i}", [dinp, HC], BF16)
        Wd[f"attB{i}"] = din(f"attB{i}", [128, HC])
        for nm in ("bT", "gnwT", "gnbT", "gnmT"):
            Wd[f"{nm}{i}"] = din(f"{nm[:-1]}T{i}", [128, HC // 128])
        Wd[f"S{i}"] = din(f"S{i}", [4, HC])
    aW1_d = din("aW1", [128, 128], BF16)
    ab1B_d = din("ab1B", [128, 128])
    aW2B_d = din("aW2B", [128, 128])
    ab2B_d = din("ab2B", [128, 1])
    iota128_d = din("iota128", [128, 128])
    iota64_d = din("iota64", [128, 64])
    ident_d = din("ident", [128, 128])
    cv15_d = din("cv15", [128, 128])
    ones_col_d = din("ones_col", [128, 1], BF16)

    out_d = nc.dram_tensor("out", [B, 128], F32, kind="ExternalOutput")

    with tile.TileContext(nc) as tc:
        with (
            tc.tile_pool(name="dram", bufs=1, space="DRAM") as dram,
            tc.tile_pool(name="const", bufs=1) as cpool,
            tc.tile_pool(name="persist", bufs=1) as ppool,
            tc.tile_pool(name="work", bufs=3) as wpool,
            tc.tile_pool(name="small", bufs=4) as spool,
            tc.tile_pool(name="ps", bufs=2, space="PSUM") as ps,
        ):
            tblL = {}
            tblR = {}
            agin = {}
            for li, (dn, dinp, HC, C) in enumerate(LAYERS):
                tblL[li] = dram.tile([NTOT, HC], BF16, addr_space="Shared",
                                     name=f"tblL{li}")
                tblR[li] = dram.tile([NPAD, HC], BF16, name=f"tblR{li}")
                agin[li] = dram.tile([NPAD, HC], BF16, name=f"agin{li}")
            stats_in = {}
            stats_out = {}
            for li in range(3):
                stats_in[li] = dram.tile([128, 16], F32, name=f"stats_in{li}")
                stats_out[li] = dram.tile([128, 16], F32, addr_space="Shared",
                                          name=f"stats_out{li}")
            pool_in = dram.tile([B, 129], F32, name="pool_in")
            pool_out = dram.tile([B, 129], F32, addr_space="Shared",
                                 name="pool_out")

            def cload(dten, shape, dt=F32, name="c"):
                t = cpool.tile(shape, dt, name=name)
                nc.sync.dma_start(t[:], dten[:])
                return t

            srcg = cload(srcg_d, [128, NBLK * TB], I32, "srcg_sb")
            dstc = cload(dstc_d, [128, NBLK * TB], name="dstc_sb")
            vald = cload(vald_d, [128, NBLK * TB], name="vald_sb")
            gbt = cload(gbt_d, [128, NT], name="gbt_sb")
            nvt = cload(nvt_d, [128, NT], name="nvt_sb")
            iota128 = cload(iota128_d, [128, 128], name="iota128_sb")
            iota64 = cload(iota64_d, [128, 64], name="iota64_sb")
            ident = cload(ident_d, [128, 128], name="ident_sb")
            cv15 = cload(cv15_d, [128, 128], name="cv15_sb")
            ones_col = cload(ones_col_d, [128, 1], BF16, "ones_sb")
            aW1 = cload(aW1_d, [128, 128], BF16, "aW1_sb")
            ab1B = cload(ab1B_d, [128, 128], name="ab1B_sb")
            aW2B = cload(aW2B_d, [128, 128], name="aW2B_sb")
            ab2B = cload(ab2B_d, [128, 1], name="ab2B_sb")
            identB = cpool.tile([128, 128], BF16, name="identB_sb")
            nc.vector.tensor_copy(identB[:], ident[:])
            # iota replicated TB times (for batched mask gen), bf16 (0..127 exact)
            iota_rep = cpool.tile([128, TB * 128], BF16, name="iota_rep_sb")
            for q in range(TB):
                nc.vector.tensor_copy(iota_rep[:, q * 128:(q + 1) * 128], iota128[:])

            # persistent transposed activations (bf16)
            xTn = {1: [], 2: [], 3: []}
            for i in (1, 2, 3):
                chn = LAYERS[i - 1][2] // 128
                for cc in range(chn):
                    t = ppool.tile([128, NPAD], BF16, name=f"xT{i}_{cc}")
                    xTn[i].append(t)

            ExpF = mybir.ActivationFunctionType.Exp
            PreluF = mybir.ActivationFunctionType.Prelu
            SquareF = mybir.ActivationFunctionType.Square
            SqrtF = mybir.ActivationFunctionType.Sqrt
            CopyF = mybir.ActivationFunctionType.Copy

            # ================= per-layer pipeline =================
            for li, (dn, dinp, HC, C) in enumerate(LAYERS):
                i = li + 1
                CH = HC // 128
                CHI = dinp // 128

                wl_sb = []
                wr_sb = []
                for ci in range(CHI):
                    wl = wpool.tile([128, HC], BF16, tag="wl", name=f"wl{i}_{ci}",
                                    bufs=8)
                    nc.sync.dma_start(wl[:], Wd[f"Wl{i}"][ci * 128:(ci + 1) * 128, :])
                    wl_sb.append(wl)
                    wr = wpool.tile([128, HC], BF16, tag="wr", name=f"wr{i}_{ci}",
                                    bufs=8)
                    nc.sync.dma_start(wr[:], Wd[f"Wr{i}"][ci * 128:(ci + 1) * 128, :])
                    wr_sb.append(wr)
                attB = wpool.tile([128, HC], F32, tag="attB", name=f"attB{i}_sb",
                                  bufs=1)
                nc.sync.dma_start(attB[:], Wd[f"attB{i}"][:])
                attBc = wpool.tile([128, HC], BF16, tag="attBc", name=f"attBc{i}",
                                   bufs=1)
                nc.vector.tensor_copy(attBc[:], attB[:])
                attR = wpool.tile([128, TBH * HC], BF16, tag="attR",
                                  name=f"attR{i}", bufs=1)
                for q in range(TBH):
                    nc.vector.tensor_copy(attR[:, q * HC:(q + 1) * HC], attBc[:])
                bT = spool.tile([128, CH], F32, tag="bT", name=f"bT{i}_sb", bufs=1)
                nc.sync.dma_start(bT[:], Wd[f"bT{i}"][:])
                Ssel = spool.tile([4, HC], F32, tag="Ssel", name=f"S{i}_sb", bufs=1)
                nc.sync.dma_start(Ssel[:], Wd[f"S{i}"][:])

                # ---- matmul phase: xl/xr rows (bf16) for all node tiles
                def emit_rows(nt, ps_xl, ps_xr):
                    xl_sb = wpool.tile([128, HC], BF16, tag="xlrow", name="xl_sb")
                    xr_sb = wpool.tile([128, HC], BF16, tag="xrrow", name="xr_sb")
                    nc.vector.tensor_copy(xl_sb[:], ps_xl[:])
                    nc.vector.tensor_copy(xr_sb[:], ps_xr[:])
                    nc.sync.dma_start(agin[li][nt * 128:(nt + 1) * 128, :], xl_sb[:])
                    nc.sync.dma_start(tblR[li][nt * 128:(nt + 1) * 128, :], xr_sb[:])

                if li == 0:
                    for g in range(NT // 2):
                        pxl = [ps.tile([128, HC], F32, tag="big", name="ps_xl",
                                       bufs=4) for _ in range(2)]
                        pxr = [ps.tile([128, HC], F32, tag="big", name="ps_xr",
                                       bufs=4) for _ in range(2)]
                        for ci in range(CHI):
                            lhsg = wpool.tile([128, 256], BF16, tag="xt0",
                                              name="xt0_t", bufs=4)
                            nc.sync.dma_start(
                                lhsg[:], xT0_d[ci * 128:(ci + 1) * 128,
                                               g * 256:(g + 1) * 256])
                            for t in range(2):
                                st = (ci == 0)
                                sp = (ci == CHI - 1)
                                nc.tensor.matmul(
                                    pxl[t][:], lhsT=lhsg[:, t * 128:(t + 1) * 128],
                                    rhs=wl_sb[ci][:], start=st, stop=sp)
                                nc.tensor.matmul(
                                    pxr[t][:], lhsT=lhsg[:, t * 128:(t + 1) * 128],
                                    rhs=wr_sb[ci][:], start=st, stop=sp)
                        for t in range(2):
                            emit_rows(g * 2 + t, pxl[t], pxr[t])
                else:
                    for nt in range(NT):
                        ps_xl = ps.tile([128, HC], F32, tag="big", name="ps_xl",
                                        bufs=4)
                        ps_xr = ps.tile([128, HC], F32, tag="big", name="ps_xr",
                                        bufs=4)
                        for ci in range(CHI):
                            lhs = xTn[li][ci][:, nt * 128:(nt + 1) * 128]
                            st = (ci == 0)
                            sp = (ci == CHI - 1)
                            nc.tensor.matmul(ps_xl[:], lhsT=lhs, rhs=wl_sb[ci][:],
                                             start=st, stop=sp)
                            nc.tensor.matmul(ps_xr[:], lhsT=lhs, rhs=wr_sb[ci][:],
                                             start=st, stop=sp)
                        emit_rows(nt, ps_xl, ps_xr)

                nc.gpsimd.collective_compute(
                    "AllGather", mybir.AluOpType.bypass, replica_groups=RG,
                    ins=[agin[li].opt()], outs=[tblL[li].opt()],
                )

                # ---- edge phase
                for _rep in range(EDGE_REPS):
                  for b in range(NBLK):
                    acc_ps = [ps.tile([128, 128], F32, tag="big",
                                      name=f"acc_ps{cc}", bufs=4)
                              for cc in range(CH)]
                    z_ps = ps.tile([4, 128], F32, tag="z", name="z_ps", bufs=1)
                    xr_blk = wpool.tile([128, HC], BF16, tag="xrblk",
                                        name="xr_blk", bufs=2)
                    nc.sync.dma_start(xr_blk[:],
                                      tblR[li][b * 128:(b + 1) * 128, :])
                    # batched mask generation for the whole block
                    mask_blk = wpool.tile([128, TB, 128], BF16, tag="maskblk",
                                          name="mask_blk", bufs=2)
                    nc.vector.tensor_tensor(
                        out=mask_blk[:],
                        in0=dstc[:, b * TB:(b + 1) * TB].to_broadcast(
                            [128, TB, 128]),
                        in1=iota_rep[:].rearrange("p (t q) -> p t q", q=128),
                        op=mybir.AluOpType.is_equal)
                    logit_blk = wpool.tile([128, TB * 4], F32, tag="logitblk",
                                           name="logit_blk", bufs=2)
                    gl_tiles = {}
                    for hh, (j0, w) in enumerate(HALVES):
                        gl = wpool.tile([128, TBH, HC], BF16, tag="gl",
                                        name="gl", bufs=4)
                        gl_tiles[hh] = gl
                        tsum = wpool.tile([128, TBH, HC], BF16, tag="ts",
                                          name="tsum", bufs=2)
                        for jj in range(w):
                            j = j0 + jj
                            s = b * TB + j
                            nc.gpsimd.indirect_dma_start(
                                out=gl[:, jj, :], out_offset=None, in_=tblL[li][:],
                                in_offset=bass.IndirectOffsetOnAxis(
                                    ap=srcg[:, s:s + 1], axis=0))
                            maskT_ps = ps.tile([128, 128], BF16, tag="zrb",
                                               name="maskT_ps", bufs=2)
                            nc.tensor.transpose(maskT_ps[:], mask_blk[:, j, :],
                                                identB[:])
                            maskT = wpool.tile([128, 128], BF16, tag="maskT",
                                               name="maskT")
                            nc.scalar.activation(maskT[:], maskT_ps[:], CopyF)
                            grexp = ps.tile([128, HC], F32, tag="grexp",
                                            name="grexp", bufs=1)
                            nc.tensor.matmul(grexp[:], lhsT=maskT[:],
                                             rhs=xr_blk[:], start=True, stop=True)
                            nc.vector.tensor_add(tsum[:, jj, :], gl[:, jj, :],
                                                 grexp[:])
                        # batched logit chain over the half
                        tflat = tsum[:, :w, :].rearrange("p w h -> p (w h)")
                        nc.scalar.activation(tflat, tflat, PreluF, alpha=0.2)
                        nc.vector.tensor_mul(tflat, tflat, attR[:, :w * HC])
                        nc.vector.tensor_reduce(
                            logit_blk[:, j0 * 4:(j0 + w) * 4],
                            tsum[:, :w, :].rearrange("p w (h c) -> p w h c", h=4),
                            axis=mybir.AxisListType.X, op=mybir.AluOpType.add)
                    pex_blk = wpool.tile([128, TB * 4], F32, tag="pexblk",
                                         name="pex_blk", bufs=2)
                    nc.scalar.activation(pex_blk[:], logit_blk[:], ExpF)
                    pv_blk = wpool.tile([128, TB, 4], BF16, tag="pvblk",
                                        name="pv_blk", bufs=2)
                    nc.vector.tensor_tensor(
                        out=pv_blk[:],
                        in0=pex_blk[:].rearrange("p (t h) -> p t h", h=4),
                        in1=vald[:, b * TB:(b + 1) * TB].to_broadcast(
                            [128, TB, 4]),
                        op=mybir.AluOpType.mult)
                    for hh, (j0, w) in enumerate(HALVES):
                        gl = gl_tiles[hh]
                        # glp = gl * p  (in place on gl)
                        nc.vector.tensor_tensor(
                            out=gl[:, :w, :].rearrange("p w (h c) -> p w h c", h=4),
                            in0=gl[:, :w, :].rearrange("p w (h c) -> p w h c", h=4),
                            in1=pv_blk[:, j0:j0 + w, :].to_broadcast(
                                [128, w, 4, C]),
                            op=mybir.AluOpType.mult)
                        for jj in range(w):
                            j = j0 + jj
                            st = (j == 0)
                            sp = (j == TB - 1)
                            nc.tensor.matmul(
                                z_ps[:],
                                lhsT=pv_blk[:, j, :], rhs=mask_blk[:, j, :],
                                start=st, stop=sp)
                            for cc in range(CH):
                                nc.tensor.matmul(
                                    acc_ps[cc][:],
                                    lhsT=gl[:, jj, cc * 128:(cc + 1) * 128],
                                    rhs=mask_blk[:, j, :], start=st, stop=sp)
                    # ---- post-block: z reciprocal, scale, bias
                    zrec = spool.tile([4, 128], F32, tag="zrec", name="zrec")
                    nc.vector.tensor_scalar_add(zrec[:], z_ps[:], 1e-16)
                    nc.vector.reciprocal(zrec[:], zrec[:])
                    for cc in range(CH):
                        zrb_ps = ps.tile([128, 128], F32, tag="z", name="zrb_ps",
                                         bufs=1)
                        nc.tensor.matmul(zrb_ps[:],
                                         lhsT=Ssel[:, cc * 128:(cc + 1) * 128],
                                         rhs=zrec[:], start=True, stop=True)
                        zrb = wpool.tile([128, 128], F32, tag="zrb_sb",
                                         name="zrb_sb")
                        nc.vector.tensor_copy(zrb[:], zrb_ps[:])
                        dstv = xTn[i][cc][:, b * 128:(b + 1) * 128]
                        nc.vector.tensor_mul(dstv, acc_ps[cc][:], zrb[:])
                        nc.vector.tensor_scalar_add(dstv, dstv, bT[:, cc:cc + 1])
                        if b == NBLK - 1:
                            nc.vector.tensor_mul(dstv, dstv, cv15[:])

                # ---- GraphNorm stats: S1 cols 0..CH-1, S2 cols 8..8+CH-1
                stats = spool.tile([128, 16], F32, tag="stats", name="stats")
                nc.vector.memset(stats[:], 0.0)
                sq = wpool.tile([128, NPAD], BF16, tag="sq", name="sq", bufs=1)
                for cc in range(CH):
                    nc.vector.tensor_reduce(
                        stats[:, cc:cc + 1], xTn[i][cc][:],
                        axis=mybir.AxisListType.X, op=mybir.AluOpType.add)
                    nc.scalar.activation(sq[:], xTn[i][cc][:], SquareF)
                    nc.vector.tensor_reduce(
                        stats[:, 8 + cc:8 + cc + 1], sq[:],
                        axis=mybir.AxisListType.X, op=mybir.AluOpType.add)
                nc.sync.dma_start(stats_in[li][:], stats[:])
                nc.gpsimd.collective_compute(
                    "AllReduce", mybir.AluOpType.add, replica_groups=RG,
                    ins=[stats_in[li].opt()], outs=[stats_out[li].opt()],
                )
                gstats = spool.tile([128, 16], F32, tag="gstats", name="gstats")
                nc.sync.dma_start(gstats[:], stats_out[li][:])

                gnw = spool.tile([128, CH], F32, tag="gnw", name="gnw")
                gnb = spool.tile([128, CH], F32, tag="gnb", name="gnb")
                gnm = spool.tile([128, CH], F32, tag="gnm", name="gnm")
                nc.sync.dma_start(gnw[:], Wd[f"gnwT{i}"][:])
                nc.sync.dma_start(gnb[:], Wd[f"gnbT{i}"][:])
                nc.sync.dma_start(gnm[:], Wd[f"gnmT{i}"][:])

                mn = spool.tile([128, CH], F32, tag="mn", name="mn")
                e2 = spool.tile([128, CH], F32, tag="e2", name="e2")
                nc.vector.tensor_scalar_mul(mn[:], gstats[:, :CH], 1.0 / N)
                nc.vector.tensor_scalar_mul(e2[:], gstats[:, 8:8 + CH], 1.0 / N)
                msm = spool.tile([128, CH], F32, tag="msm", name="msm")
                nc.vector.tensor_mul(msm[:], mn[:], gnm[:])
                t1 = spool.tile([128, CH], F32, tag="t1", name="t1")
                nc.vector.tensor_scalar_mul(t1[:], mn[:], 2.0)
                nc.vector.tensor_sub(t1[:], t1[:], msm[:])
                nc.vector.tensor_mul(t1[:], t1[:], msm[:])
                var = spool.tile([128, CH], F32, tag="var", name="var")
                nc.vector.tensor_sub(var[:], e2[:], t1[:])
                nc.vector.tensor_scalar_add(var[:], var[:], 1e-5)
                sd = spool.tile([128, CH], F32, tag="sd", name="sd")
                nc.scalar.activation(sd[:], var[:], SqrtF)
                istd = spool.tile([128, CH], F32, tag="istd", name="istd")
                nc.vector.reciprocal(istd[:], sd[:])
                A = spool.tile([128, CH], F32, tag="A", name="A")
                nc.vector.tensor_mul(A[:], gnw[:], istd[:])
                B2 = spool.tile([128, CH], F32, tag="B2", name="B2")
                nc.vector.tensor_mul(B2[:], A[:], msm[:])
                nc.vector.tensor_sub(B2[:], gnb[:], B2[:])
                for cc in range(CH):
                    nc.vector.tensor_scalar(
                        out=xTn[i][cc][:], in0=xTn[i][cc][:],
                        scalar1=A[:, cc:cc + 1], scalar2=B2[:, cc:cc + 1],
                        op0=mybir.AluOpType.mult, op1=mybir.AluOpType.add)
                    nc.vector.tensor_scalar_max(xTn[i][cc][:], xTn[i][cc][:], 0.0)

            # ================= pooling =================
            x3 = xTn[3][0]
            x3r = []
            for ntl in range(NT):
                tps = ps.tile([128, 128], BF16, tag="big", name="tps", bufs=4)
                nc.tensor.transpose(tps[:], x3[:, ntl * 128:(ntl + 1) * 128],
                                    identB[:])
                xr_t = ppool.tile([128, 128], BF16, name=f"x3r_{ntl}")
                nc.vector.tensor_copy(xr_t[:], tps[:])
                x3r.append(xr_t)

            gate = spool.tile([128, NT], F32, tag="gate", name="gate")
            for ntl in range(NT):
                h_ps = ps.tile([128, 128], F32, tag="big", name="h_ps", bufs=4)
                nc.tensor.matmul(h_ps[:], lhsT=x3[:, ntl * 128:(ntl + 1) * 128],
                                 rhs=aW1[:], start=True, stop=True)
                hsb = wpool.tile([128, 128], F32, tag="hsb", name="hsb")
                nc.vector.tensor_add(hsb[:], h_ps[:], ab1B[:])
                nc.vector.tensor_scalar_max(hsb[:], hsb[:], 0.0)
                nc.vector.tensor_mul(hsb[:], hsb[:], aW2B[:])
                nc.vector.tensor_reduce(
                    gate[:, ntl:ntl + 1], hsb[:],
                    axis=mybir.AxisListType.X, op=mybir.AluOpType.add)
            nc.vector.tensor_scalar_add(gate[:], gate[:], ab2B[:, 0:1])
            pgate = spool.tile([128, NT], F32, tag="pgate", name="pgate")
            pge = spool.tile([128, NT], F32, tag="pge", name="pge")
            nc.scalar.activation(pge[:], gate[:], ExpF)
            nc.vector.tensor_mul(pgate[:], pge[:], nvt[:])

            pool_ps = ps.tile([B, 128], F32, tag="z", name="pool_ps", bufs=1)
            poolz_ps = ps.tile([B, 1], F32, tag="grexp", name="poolz_ps", bufs=1)
            for ntl in range(NT):
                mb = wpool.tile([128, B], BF16, tag="mb", name="mb")
                nc.vector.tensor_tensor(
                    out=mb[:], in0=gbt[:, ntl:ntl + 1].to_broadcast([128, B]),
                    in1=iota64[:], op=mybir.AluOpType.is_equal)
                nc.vector.tensor_scalar_mul(mb[:], mb[:], pgate[:, ntl:ntl + 1])
                st = (ntl == 0)
                sp = (ntl == NT - 1)
                nc.tensor.matmul(pool_ps[:], lhsT=mb[:], rhs=x3r[ntl][:],
                                 start=st, stop=sp)
                nc.tensor.matmul(poolz_ps[:], lhsT=mb[:], rhs=ones_col[:],
                                 start=st, stop=sp)
            pool_sb = spool.tile([B, 129], F32, tag="poolsb", name="pool_sb")
            nc.vector.tensor_copy(pool_sb[:, :128], pool_ps[:])
            nc.vector.tensor_copy(pool_sb[:, 128:129], poolz_ps[:])
            nc.sync.dma_start(pool_in[:], pool_sb[:])
            nc.gpsimd.collective_compute(
                "AllReduce", mybir.AluOpType.add, replica_groups=RG,
                ins=[pool_in.opt()], outs=[pool_out.opt()],
            )
            poolg = spool.tile([B, 129], F32, tag="poolg", name="poolg")
            nc.sync.dma_start(poolg[:], pool_out[:])
            zg = spool.tile([B, 1], F32, tag="zg", name="zg")
            nc.vector.tensor_scalar_add(zg[:], poolg[:, 128:129], 1e-16)
            nc.vector.reciprocal(zg[:], zg[:])
            outv = spool.tile([B, 128], F32, tag="outv", name="outv")
            nc.vector.tensor_scalar_mul(outv[:], poolg[:, :128], zg[:, 0:1])
            nc.sync.dma_start(out_d[:], outv[:])

    nc.finalize()
    return nc


# ---------------------------------------------------------------- runner

_CACHE = {}
LAST_RESULTS = None
LAST_EXEC_NS = None


def _make_runner(t_blk):
    import jax
    import numpy as _np
    from jax.sharding import Mesh, PartitionSpec
    from jax.experimental.shard_map import shard_map
    from concourse import bass2jax, mybir as mb

    nc = _build_program(t_blk)
    bass2jax.install_neuronx_cc_hook()

    partition_name = (nc.partition_id_tensor.name
                      if nc.partition_id_tensor else None)
    in_names, out_names, out_avals, zero_outs = [], [], [], []
    for alloc in nc.m.functions[0].allocations:
        if not isinstance(alloc, mb.MemoryLocationSet):
            continue
        name = alloc.memorylocations[0].name
        if alloc.kind == "ExternalInput":
            if name != partition_name:
                in_names.append(name)
        elif alloc.kind == "ExternalOutput":
            out_names.append(name)
            shape = tuple(alloc.tensor_shape)
            dtype = mb.dt.np(alloc.dtype)
            out_avals.append(jax.core.ShapedArray(shape, dtype))
            zero_outs.append(_np.zeros(shape, dtype))
    n_params = len(in_names)
    all_in_names = list(in_names) + list(out_names)
    if partition_name is not None:
        all_in_names.append(partition_name)

    def _body(*args):
        operands = list(args)
        if partition_name is not None:
            operands.append(bass2jax.partition_id_tensor())
        outs = bass2jax._bass_exec_p.bind(
            *operands,
            out_avals=tuple(out_avals),
            in_names=tuple(all_in_names),
            out_names=tuple(out_names),
            lowering_input_output_aliases=(),
            sim_require_finite=False,
            sim_require_nnan=False,
            nc=nc,
        )
        return tuple(outs)

    devices = jax.devices()[:NCORES]
    mesh = Mesh(_np.asarray(devices), ("core",))
    in_specs = (PartitionSpec("core"),) * (n_params + len(out_names))
    out_specs = (PartitionSpec("core"),) * len(out_names)
    sharded = jax.jit(
        shard_map(_body, mesh=mesh, in_specs=in_specs, out_specs=out_specs,
                  check_rep=False),
        keep_unused=True,
    )
    return dict(nc=nc, sharded=sharded, in_names=in_names,
                out_names=out_names, out_avals=out_avals, zero_outs=zero_outs)


def _get_runner(t_blk):
    if t_blk not in _CACHE:
        _CACHE[t_blk] = _make_runner(t_blk)
    return _CACHE[t_blk]


def _concat_inputs(runner, in_maps):
    import numpy as _np
    per_core = [[_np.ascontiguousarray(m[nm]) for nm in runner["in_names"]]
                for m in in_maps]
    concat_in = [
        _np.concatenate([per_core[c][i] for c in range(NCORES)], axis=0)
        for i in range(len(runner["in_names"]))
    ]
    concat_zeros = [
        _np.zeros((NCORES * z.shape[0], *z.shape[1:]), z.dtype)
        for z in runner["zero_outs"]
    ]
    return concat_in, concat_zeros


def _run(runner, concat_in, concat_zeros):
    import numpy as _np
    out_arrs = runner["sharded"](*concat_in, *concat_zeros)
    results = []
    outs = [_np.asarray(a) for a in out_arrs]
    for c in range(NCORES):
        results.append({
            name: outs[i].reshape(NCORES, *runner["out_avals"][i].shape)[c]
            for i, name in enumerate(runner["out_names"])
        })
    return results


def _build_in_maps(inputs):
    per_core, shared, t_blk = _host_prep(inputs)
    in_maps = []
    for k in range(NCORES):
        m = {
            "srcg": per_core["srcg"][k], "dstc": per_core["dstc"][k],
            "vald": per_core["vald"][k], "xT0": per_core["xT0"][k],
            "gbt": per_core["gbt"][k], "nvt": per_core["nvt"][k],
        }
        m.update(shared)
        in_maps.append(m)
    return in_maps, t_blk


def kernel(**inputs):
    in_maps, t_blk = _build_in_maps(inputs)
    runner = _get_runner(t_blk)
    concat_in, concat_zeros = _concat_inputs(runner, in_maps)
    results = _run(runner, concat_in, concat_zeros)
    global LAST_RESULTS
    LAST_RESULTS = results
    return results[0]["out"]


def time_kernel(inputs, iters=8):
    import time as _time
    import jax
    in_maps, t_blk = _build_in_maps(inputs)
    runner = _get_runner(t_blk)
    concat_in, concat_zeros = _concat_inputs(runner, in_maps)
    ci = [jax.device_put(a) for a in concat_in]
    r = runner["sharded"](*ci, *concat_zeros)
    jax.block_until_ready(r)
    t0 = _time.time()
    r = runner["sharded"](*ci, *concat_zeros)
    jax.block_until_ready(r)
    t1 = _time.time()
    rs = []
    t2 = _time.time()
    for _ in range(iters):
        rs.append(runner["sharded"](*ci, *concat_zeros))
    jax.block_until_ready(rs)
    t3 = _time.time()
    one = t1 - t0
    many = t3 - t2
    est = (many - one) / (iters - 1)
    return dict(one_s=one, many_s=many, est_exec_s=est)


if __name__ == "__main__":
    import reference
    inputs = {k: np.asarray(v) for k, v in reference.setup_inputs().items()}
    out = kernel(**inputs)
    exp = np.asarray(reference.reference(**inputs))
    err = np.abs(out - exp).max() / (np.abs(exp).max() + 1e-12)
    print("rel err:", err)
